# revision 24
# baseline (speedup 1.0000x reference)
"""Trainium2 Bass kernel for nn_AbstractAttention (dense transformer attention
with RoPE, B=2 S=2048 D=4096 H=32), tensor-parallel over heads on 8 cores.

Single-pass bf16 design (rel tolerance is 2e-2; end-to-end bf16 gives ~5e-3):
every logical matmul is ONE bf16 PE pass (vs the 3-term fp32-split baseline),
and all matmuls are emitted in "flipped" orientations so no PE transposes are
needed anywhere:

  q^T/k^T [dl, t]  = (wT chunk)^T @ x^T      (weight stationary)
  v [t, e]         = (x^T chunk)^T @ wvT     (x stationary)
  s^T [k, q]       = (k^T chunk)^T @ q^T     (k stationary)
  ctx^T [hd, q]    = (v chunk)^T @ p^T       (v stationary)  <- accumulates
  out^T [e, t]     = (woT chunk)^T @ ctx^T   (wo stationary)

RoPE is applied on the [hd-partition, token-free] layout: the pair swap is a
constant 128x128 permutation matmul on the PE; cos/sin become full [128, S]
tables (host-precomputed, sign+mask folded in) so the mix is 3 DVE ops.

Softmax skips max-subtraction (scores are O(1)); the denominator is a
Pool-engine accumulation of exp tiles + a ones-row matmul partition-reduce,
and normalization happens on the small context (not the probs) via a rank-1
broadcast matmul of the reciprocal.

Host: shards weights by head, preps bf16 transposed layouts, sums the 8
partial out^T tensors in fp32 and transposes back.
"""

import math

import numpy as np

import concourse.bacc as bacc
import concourse.mybir as mybir
from concourse.bass_utils import run_bass_kernel_spmd
from concourse.tile import TileContext

P = 128
F32 = mybir.dt.float32
BF16 = mybir.dt.bfloat16
ALU = mybir.AluOpType
ACTF = mybir.ActivationFunctionType

# set by test.py for profiling; grading path leaves these alone
TRACE = False
TRACE_DIR = None
LAST_RESULT = [None]

ZERO, MIXED, SKIP = 0, 1, 2


def _classify_blocks(maskT, S, QT):
    """maskT: [S, S] (k, q). Diagonal-straddling blocks get explicit tiles."""
    nqt, nkc = S // QT, S // P
    kinds = [[ZERO] * nkc for _ in range(nqt)]
    tiles = []
    index = {}
    for qt in range(nqt):
        for kc in range(nkc):
            sub = maskT[kc * P : (kc + 1) * P, qt * QT : (qt + 1) * QT]
            if np.all(sub == 0.0):
                kinds[qt][kc] = ZERO
            elif np.all(np.isneginf(sub) | (sub < -1e30)):
                kinds[qt][kc] = SKIP
            else:
                kinds[qt][kc] = MIXED
                index[(qt, kc)] = len(tiles)
                # pre-scale by sqrt(HD): kernel computes exp((S + m)/sqrt(HD))
                m = np.maximum(sub.astype(np.float64) * math.sqrt(P), -1e30)
                tiles.append(m.astype(np.float32))
    if tiles:
        mixed = np.ascontiguousarray(np.stack(tiles)).astype(np.float32)
    else:
        mixed = np.zeros((1, P, QT), dtype=np.float32)
    return kinds, mixed, index


def _p2_tail(nc, psden, apool, cpool, ones_mat, accA, accB, ctx_ps, ctx_scr_r,
             h, tcol):
    """Deferred per-(b,h,qt) denominator reduce + context normalization."""
    P = 128
    F32 = mybir.dt.float32
    BF16 = mybir.dt.bfloat16
    ALU = mybir.AluOpType
    nc.vector.tensor_tensor(accA, accA, accB, ALU.add)
    den = psden.tile([P, 512], F32, tag="den")
    nc.tensor.matmul(den, ones_mat, accA, start=True, stop=True)
    rec = apool.tile([P, 512], F32, tag="rec")
    scr = apool.tile([P, 512], F32, tag="rscr")
    nc.vector.reciprocal_approx_accurate(rec, den, scr)
    csb = cpool.tile([P, 512], BF16, tag="csb")
    nc.vector.tensor_tensor(csb, ctx_ps, rec, ALU.mult)
    nc.sync.dma_start(ctx_scr_r[:, h, tcol * 512 : (tcol + 1) * 512], csb)


def _build(B, S, D, HL, kinds, mixed_index, n_mixed):
    """Per-core Bass program. HL local heads, DL=HL*128 local dims."""
    DL = HL * P
    T = B * S
    KD = D // P        # 32 K-chunks of the model dim
    NTG = T // 512     # 8 token groups of 512
    NKC = S // P       # 16 key chunks per batch
    NQT = S // 512     # 4 query tiles per batch
    inv_sqrt_d = 1.0 / math.sqrt(P)

    nc = bacc.Bacc(None, target_bir_lowering=False)

    xT = nc.declare_dram_parameter("xT", [D, T], BF16, isOutput=False)
    wqT = nc.declare_dram_parameter("wqT", [D, DL], BF16, isOutput=False)
    wkT = nc.declare_dram_parameter("wkT", [D, DL], BF16, isOutput=False)
    wvT = nc.declare_dram_parameter("wvT", [D, DL], BF16, isOutput=False)
    woT = nc.declare_dram_parameter("woT", [DL, D], BF16, isOutput=False)
    cosE = nc.declare_dram_parameter("cosE", [P, S], BF16, isOutput=False)
    sinS = nc.declare_dram_parameter("sinS", [P, S], BF16, isOutput=False)
    pswap = nc.declare_dram_parameter("pswap", [P, P], BF16, isOutput=False)
    mtiles = nc.declare_dram_parameter(
        "mask_tiles", [max(n_mixed, 1), P, 512], F32, isOutput=False
    )
    outT = nc.declare_dram_parameter("outT", [D, T], BF16, isOutput=True)

    ts = lambda i, s: slice(i * s, (i + 1) * s)

    xT_r = xT.ap().rearrange("(o p) t -> p o t", p=P)
    woT_r = woT.ap().rearrange("(o p) e -> p o e", p=P)
    outT_r = outT.ap().rearrange("(o p) t -> p o t", p=P)

    with TileContext(nc) as tc:
        with (
            tc.tile_pool(name="res", bufs=1) as res,
            tc.tile_pool(name="consts", bufs=1) as consts,
            tc.tile_pool(name="dram", bufs=1, space="DRAM") as dram,
        ):
            # resident q^T/k^T (roped, bf16) and v
            qres = res.tile([P, HL, T], BF16)
            kres = res.tile([P, HL, T], BF16)
            vres = res.tile([P, T // P, DL], BF16)  # [k-in-chunk, t-chunk, (h,hd)]
            ctx_scr = dram.tile([DL, T], BF16, tag="ctxs", name="ctxs")
            ctx_scr_r = ctx_scr.rearrange("(o p) t -> p o t", p=P)

            ones_mat = consts.tile([P, P], F32)
            nc.vector.memset(ones_mat, 1.0)

            # ============ Phase 1: projections, order v -> q -> k ============
            # One rotating weights pool (quarter-tiles for fine-grained DMA
            # deps) so the next projection's weights prefetch during the
            # current one's matmuls.
            with (
                tc.tile_pool(name="p1c", bufs=1) as p1c,
                tc.tile_pool(name="wpool", bufs=2) as wpool,
                tc.tile_pool(name="xpool", bufs=3) as xpool,
                tc.tile_pool(name="rp", bufs=3) as rp,
                tc.tile_pool(name="psacc", bufs=6, space="PSUM") as psacc,
                tc.tile_pool(name="pssw", bufs=2, space="PSUM") as pssw,
            ):
                cos_sb = p1c.tile([P, S], BF16)
                nc.gpsimd.dma_start(cos_sb, cosE.ap())
                sin_sb = p1c.tile([P, S], BF16)
                nc.gpsimd.dma_start(sin_sb, sinS.ap())
                psw_sb = p1c.tile([P, P], BF16)
                nc.gpsimd.dma_start(psw_sb, pswap.ap())

                def load_w(src):
                    # scalar-queue issue: keeps x tiles (sync queue) unblocked
                    qtr = []
                    r = src.ap().rearrange("(o p) n -> p o n", p=P)
                    for kq in range(4):
                        t = wpool.tile([P, 8, DL], BF16, tag=f"w{kq}")
                        nc.scalar.dma_start(t, r[:, ts(kq, 8), :])
                        qtr.append(t)
                    return qtr

                wv_sb = load_w(wvT)
                wq_sb = load_w(wqT)

                # --- v pass (x stationary) ---
                for tg in range(NTG):
                    vbanks = [
                        psacc.tile([P, DL], F32, tag="acc", name=f"vac{tg}{u}")
                        for u in range(4)
                    ]
                    for kq in range(4):
                        xt = xpool.tile([P, 8, 512], BF16, tag="xt")
                        nc.sync.dma_start(xt, xT_r[:, ts(kq, 8), ts(tg, 512)])
                        for u in range(4):
                            for kc in range(8):
                                nc.tensor.matmul(
                                    vbanks[u],
                                    xt[:, kc, ts(u, P)],
                                    wv_sb[kq][:, kc, :],
                                    start=(kq == 0 and kc == 0),
                                    stop=(kq == 3 and kc == 7),
                                )
                    for u in range(4):
                        nc.scalar.copy(vres[:, tg * 4 + u, :], vbanks[u])

                # --- q/k passes (weight stationary + rope) ---
                wk_sb = load_w(wkT)
                for proj in range(2):
                    w_sb = wq_sb if proj == 0 else wk_sb
                    dst = qres if proj == 0 else kres
                    for tg in range(NTG):
                        banks = [
                            psacc.tile([P, 512], F32, tag="acc", name=f"ac{proj}{tg}{d}")
                            for d in range(HL)
                        ]
                        for kq in range(4):
                            xt = xpool.tile([P, 8, 512], BF16, tag="xt")
                            nc.sync.dma_start(
                                xt, xT_r[:, ts(kq, 8), ts(tg, 512)]
                            )
                            for d in range(HL):
                                for kc in range(8):
                                    nc.tensor.matmul(
                                        banks[d],
                                        w_sb[kq][:, kc, ts(d, P)],
                                        xt[:, kc, :],
                                        start=(kq == 0 and kc == 0),
                                        stop=(kq == 3 and kc == 7),
                                    )
                        stg = tg % (S // 512)  # rope position repeats per batch
                        for d in range(HL):
                            # PSUM -> SBUF bf16 raw copy (frees the acc bank)
                            qraw = rp.tile([P, 512], BF16, tag="qraw")
                            nc.scalar.copy(qraw, banks[d])
                            # pair-swap via constant permutation matmul
                            qsw = pssw.tile([P, 512], F32, tag="sw")
                            nc.tensor.matmul(qsw, psw_sb, qraw, start=True, stop=True)
                            # rope mix: dst = raw*cos + swap*sin  (3 DVE ops)
                            dslc = dst[:, d, ts(tg, 512)]
                            nc.vector.tensor_tensor(
                                dslc, qraw, cos_sb[:, ts(stg, 512)], ALU.mult
                            )
                            tmp = rp.tile([P, 512], BF16, tag="rtmp")
                            nc.vector.tensor_tensor(
                                tmp, qsw, sin_sb[:, ts(stg, 512)], ALU.mult
                            )
                            nc.vector.tensor_tensor(dslc, dslc, tmp, ALU.add)

            # ============ Phase 2: attention ============
            active = [
                [kc for kc in range(NKC) if kinds[qt][kc] != SKIP]
                for qt in range(NQT)
            ]
            with tc.tile_pool(name="wop", bufs=1) as wop:
                # wo fully resident; DMA rides out during phase 2
                wo_sb = wop.tile([P, HL, D], BF16, tag="wo3")
                nc.scalar.dma_start(wo_sb, woT_r)

                with (
                    tc.tile_pool(name="mpool", bufs=1) as mpool,
                    tc.tile_pool(name="ppool", bufs=4) as ppool,
                    tc.tile_pool(name="apool", bufs=2) as apool,
                    tc.tile_pool(name="cpool", bufs=2) as cpool,
                    tc.tile_pool(name="pssc", bufs=3, space="PSUM") as pssc,
                    tc.tile_pool(name="psctx", bufs=2, space="PSUM") as psctx,
                    tc.tile_pool(name="psden", bufs=2, space="PSUM") as psden,
                ):
                    mcache = {}
                    for (qt, kc), idx in mixed_index.items():
                        mt = mpool.tile([P, 512], F32, tag=f"m{qt}_{kc}",
                                        name=f"m{qt}_{kc}")
                        nc.sync.dma_start(mt, mtiles.ap()[idx])
                        mcache[(qt, kc)] = mt

                    # body(i) emits scores/exp/PV; tail(i) (den/rec/norm/DMA)
                    # is emitted one iteration later so the den matmul never
                    # blocks the PE queue waiting on the Pool acc chain.
                    pending = None
                    for b in range(B):
                        for h in range(HL):
                            for qt in range(NQT):
                                acts = active[qt]
                                qslc = qres[:, h, ts(b * NQT + qt, 512)]
                                ctx_ps = psctx.tile([P, 512], F32, tag="ctx")
                                # two parallel f32 exp-sum chains: even kc on
                                # DVE, odd kc on Pool; merged in the tail
                                accA = apool.tile([P, 512], F32, tag="accA")
                                accB = apool.tile([P, 512], F32, tag="accB")
                                for i, kc in enumerate(acts):
                                    sps = pssc.tile([P, 512], F32, tag="sc")
                                    nc.tensor.matmul(
                                        sps,
                                        kres[:, h, b * S + kc * P : b * S + (kc + 1) * P],
                                        qslc,
                                        start=True,
                                        stop=True,
                                    )
                                    if kinds[qt][kc] == MIXED:
                                        nc.vector.tensor_tensor(
                                            sps, sps, mcache[(qt, kc)], ALU.add
                                        )
                                    pT = ppool.tile([P, 512], BF16, tag="pT")
                                    nc.scalar.activation(
                                        pT, sps, ACTF.Exp, scale=inv_sqrt_d
                                    )
                                    nc.tensor.matmul(
                                        ctx_ps,
                                        vres[:, b * NKC + kc, ts(h, P)],
                                        pT,
                                        start=(i == 0),
                                        stop=(i == len(acts) - 1),
                                    )
                                    eng = nc.vector if i % 2 == 0 else nc.gpsimd
                                    acc = accA if i % 2 == 0 else accB
                                    if i < 2:
                                        eng.tensor_copy(acc, pT)
                                    else:
                                        eng.tensor_tensor(acc, acc, pT, ALU.add)
                                if pending is not None:
                                    pending()
                                pending = (
                                    lambda b=b, h=h, qt=qt, accA=accA,
                                    accB=accB, ctx_ps=ctx_ps: _p2_tail(
                                        nc, psden, apool, cpool, ones_mat,
                                        accA, accB, ctx_ps, ctx_scr_r, h,
                                        b * NQT + qt
                                    )
                                )
                    if pending is not None:
                        pending()

                # ===== Phase 3: output projection (wo stationary) =====
                with (
                    tc.tile_pool(name="cx3", bufs=3) as cx3,
                    tc.tile_pool(name="ost", bufs=3) as ost,
                    tc.tile_pool(name="pso", bufs=8, space="PSUM") as pso,
                ):
                    for tg in range(NTG):
                        cxt = cx3.tile([P, HL, 512], BF16, tag="cx")
                        # gpsimd queue: out-writes saturate the sync queue
                        nc.gpsimd.dma_start(cxt, ctx_scr_r[:, :, ts(tg, 512)])
                        for es in range(D // P):
                            ps_o = pso.tile([P, 512], F32, tag="po")
                            for dl in range(HL):
                                nc.tensor.matmul(
                                    ps_o,
                                    wo_sb[:, dl, ts(es, P)],
                                    cxt[:, dl, :],
                                    start=(dl == 0),
                                    stop=(dl == HL - 1),
                                )
                            st = ost.tile([P, 512], BF16, tag="ost")
                            # split PSUM->SBUF copies across ACT and DVE so
                            # neither engine gates the phase-3 drain
                            if es % 2 == 0:
                                nc.scalar.copy(st, ps_o)
                            else:
                                nc.vector.tensor_copy(st, ps_o)
                            nc.sync.dma_start(
                                outT_r[:, es, ts(tg, 512)], st
                            )

    nc.finalize()
    return nc


def kernel(x, wq, wk, wv, wo, cos, sin, mask):
    B, S, D = x.shape
    H = D // P
    NCORES = 8
    HL = H // NCORES
    DL = HL * P
    T = B * S

    import ml_dtypes

    BF = ml_dtypes.bfloat16

    x = np.asarray(x, dtype=np.float32)
    xT = np.ascontiguousarray(x.reshape(T, D).T).astype(BF)
    cos = np.asarray(cos, dtype=np.float32)
    sin = np.asarray(sin, dtype=np.float32)

    # rope tables on [hd-partition, token-free] layout
    cosE = np.repeat(cos.T, 2, axis=0).astype(BF)          # [128, S]
    sinS = np.empty((P, S), dtype=np.float32)              # signed sin
    sinS[0::2] = -sin.T
    sinS[1::2] = sin.T
    sinS = sinS.astype(BF)
    pswap = np.zeros((P, P), dtype=np.float32)
    for r in range(P):
        pswap[r, r ^ 1] = 1.0
    pswap = pswap.astype(BF)

    maskT = np.ascontiguousarray(np.asarray(mask, dtype=np.float32)[0, 0].T)
    kinds, mixed, mixed_index = _classify_blocks(maskT, S, 512)

    nc = _build(B, S, D, HL, kinds, mixed_index, len(mixed))

    wq = np.asarray(wq, dtype=np.float32)
    wk = np.asarray(wk, dtype=np.float32)
    wv = np.asarray(wv, dtype=np.float32)
    wo = np.asarray(wo, dtype=np.float32)

    in_maps = []
    for c in range(NCORES):
        sl = slice(c * DL, (c + 1) * DL)
        m = {
            "xT": xT,
            "cosE": cosE,
            "sinS": sinS,
            "pswap": pswap,
            "mask_tiles": mixed,
            "wqT": np.ascontiguousarray(wq[sl, :].T).astype(BF),
            "wkT": np.ascontiguousarray(wk[sl, :].T).astype(BF),
            "wvT": np.ascontiguousarray(wv[sl, :].T).astype(BF),
            "woT": np.ascontiguousarray(wo[:, sl].T).astype(BF),
        }
        in_maps.append(m)

    kwargs = {}
    if TRACE:
        kwargs = {"trace": True}
        if TRACE_DIR:
            kwargs["tmpdir"] = TRACE_DIR
    res = run_bass_kernel_spmd(nc, in_maps, core_ids=list(range(NCORES)), **kwargs)
    LAST_RESULT[0] = res

    acc = res.results[0]["outT"].astype(np.float32)
    for c in range(1, NCORES):
        acc += res.results[c]["outT"].astype(np.float32)
    return np.ascontiguousarray(acc.T).reshape(B, S, D)
